# revision 6
# baseline (speedup 1.0000x reference)
"""Distributed causal multi-head attention (QKV projection + flash attention)
for Trainium2, sharded head-parallel across 8 NeuronCores.

Problem: x[2,2048,1024] @ W[1024,3072] + b -> qkv; causal softmax attention
(16 heads, head_dim 64); output [2,2048,16,64].

Sharding: core c handles batch c//4 and the 4 heads 4*(c%4)..4*(c%4)+3.
Each core's output slice is disjoint -> no collectives.

Device kernel (per core, bf16 matmuls with fp32 PSUM accumulation):
  - host passes x pre-transposed AND pre-tiled (XT [128, 8, 2048] bf16,
    partition-major over the 8 k-blocks) and W column-sliced/reordered the
    same way, so every tensor loads with ONE large dma_start (single
    completion semaphore -> back-to-back matmul dispatch, no per-piece waits)
  - projection: qT/kT produced transposed ([head-pair 128, S]) with W as the
    stationary operand; v produced natural ([S,64] tiles) with xT stationary
  - attention per head-pair: scoresT[sk,sq] = kT.T @ qT row-packed 2 heads per
    PE pass (K=64 each, tile_position rows 0-63 / 64-127) into one 2-bank PSUM
    tile; exp on ACT with scale=1/8 (ONE op per block: full blocks use the
    contiguous [128,1024] range, diagonal blocks a strided [128,2,w] AP);
    causal via partial-width blocks + a bf16 0/1 triangular mask MULTIPLY
    (DVE, post-exp) on the diagonal 128-col window only; PV accumulates
    outT[65, sq] += v'[sk,65].T @ expT[sk,sq] where v' has a ones column
    (DVE memset) -> row 64 = softmax denominator.
  - output: unnormalized [4, 65, 2048] f32; host divides by row 64, adds the
    v bias, transposes into the full output.
"""

import numpy as np

NUM_HEAD = 16
HEAD_DIM = 64
HIDDEN = 1024
B, S = 2, 2048
N_CORES = 8
HPC = 4          # heads per core
NCH = 4          # sq chunks of 512
CHW = 512        # chunk width
NT = 16          # sk tiles of 128
KB = 8           # k-dim blocks of 128
SCALE = HEAD_DIM ** -0.5

_CACHE = {}


def _build(repeat=1):
    import concourse.bacc as bacc
    import concourse.mybir as mybir
    import concourse.tile as tile

    f32 = mybir.dt.float32
    bf16 = mybir.dt.bfloat16
    AF = mybir.ActivationFunctionType

    nc = bacc.Bacc("TRN2", target_bir_lowering=False, debug=False)

    XT = nc.dram_tensor("XT", [128, KB, S], bf16, kind="ExternalInput")
    WQK = nc.dram_tensor("WQK", [128, KB, 512], bf16, kind="ExternalInput")
    WV = nc.dram_tensor("WV", [128, KB, 256], bf16, kind="ExternalInput")
    BQKT = nc.dram_tensor("BQKT", [128, 4], f32, kind="ExternalInput")
    TRI2 = nc.dram_tensor("TRI2", [128, 256], bf16, kind="ExternalInput")
    OUT = nc.dram_tensor("OUT", [HPC, 65, S], f32, kind="ExternalOutput")

    with tile.TileContext(nc) as tc:
        with tc.tile_pool(name="const", bufs=1) as const_pool, \
             tc.tile_pool(name="qkv", bufs=1) as qkv_pool, \
             tc.tile_pool(name="xt", bufs=4) as xt_pool, \
             tc.tile_pool(name="exps", bufs=8) as exp_pool, \
             tc.tile_pool(name="outs", bufs=4) as out_pool, \
             tc.tile_pool(name="ps_sc", bufs=2, space="PSUM") as ps_sc, \
             tc.tile_pool(name="ps_pr", bufs=2, space="PSUM") as ps_pr, \
             tc.tile_pool(name="ps_pv", bufs=2, space="PSUM") as ps_pv:

            for _rep in range(repeat):
                # chunk-0 activations + weights split in half so the first
                # projection chains (kb 0-3) start after half the bytes land
                def emit_xt_dma(C):
                    lo = xt_pool.tile([128, KB // 2, CHW], bf16, tag="xt")
                    hi = xt_pool.tile([128, KB // 2, CHW], bf16, tag="xt")
                    cs = slice(C * CHW, (C + 1) * CHW)
                    nc.gpsimd.dma_start(lo[:], XT[:, 0:KB // 2, cs])
                    nc.gpsimd.dma_start(hi[:], XT[:, KB // 2:KB, cs])
                    return lo, hi

                xt_first = emit_xt_dma(0)

                wqk_lo = const_pool.tile([128, KB // 2, 512], bf16, tag="wqkl")
                wqk_hi = const_pool.tile([128, KB // 2, 512], bf16, tag="wqkh")
                wv_sb = const_pool.tile([128, KB, 256], bf16, tag="wv")
                bqk_sb = const_pool.tile([128, 4], f32, tag="bqk")
                tri_sb = const_pool.tile([128, 2, 128], bf16, tag="tri")

                nc.sync.dma_start(wqk_lo[:], WQK[:, 0:KB // 2, :])
                nc.sync.dma_start(wqk_hi[:], WQK[:, KB // 2:KB, :])
                nc.sync.dma_start(wv_sb[:], WV[:])
                nc.sync.dma_start(bqk_sb[:], BQKT[:])
                nc.sync.dma_start(tri_sb[:, 0, :], TRI2[:, 0:128])
                nc.sync.dma_start(tri_sb[:, 1, :], TRI2[:, 128:256])

                # qT2/kT2: [pair, 128 (2 heads x 64 d), S]; v: [sk-tile, head, 65]
                qT2 = qkv_pool.tile([128, 2, S], bf16, tag="qT2")
                kT2 = qkv_pool.tile([128, 2, S], bf16, tag="kT2")
                v_sb = qkv_pool.tile([128, NT, HPC, 65], bf16, tag="v")
                nc.vector.memset(v_sb[:, :, :, 64], 1.0)

                H = KB // 2

                def emit_qkT_group(C, xt, blk):
                    # col-blocks: 0,1 = q pair0/pair1; 2,3 = k pair0/pair1
                    ps = ps_pr.tile([128, CHW], f32, tag="pr")
                    for kb in range(KB):
                        w_sb = wqk_lo if kb < H else wqk_hi
                        nc.tensor.matmul(
                            ps[:],
                            w_sb[:, kb % H, blk * 128:(blk + 1) * 128],
                            xt[kb // H][:, kb % H, :],
                            start=(kb == 0), stop=(kb == KB - 1))
                    dest = (qT2 if blk < 2 else kT2)[:, blk % 2,
                                                     C * CHW:(C + 1) * CHW]
                    nc.vector.tensor_scalar_add(dest, ps[:],
                                                bqk_sb[:, blk:blk + 1])

                def emit_v_group(C, xt, rt):
                    t = C * 4 + rt
                    psv = ps_pr.tile([128, 256], f32, tag="pr")
                    for kb in range(KB):
                        nc.tensor.matmul(
                            psv[:],
                            xt[kb // H][:, kb % H, rt * 128:(rt + 1) * 128],
                            wv_sb[:, kb, :],
                            start=(kb == 0), stop=(kb == KB - 1))
                    nc.vector.tensor_copy(v_sb[:, t, :, 0:64], psv[:])

                def proj_pair(C, xt, p):
                    # groups needed by pair p's attention: q blk p, k blk 2+p,
                    # plus (for p==0) all v tiles of this chunk
                    emit_qkT_group(C, xt, p)
                    emit_qkT_group(C, xt, 2 + p)
                    if p == 0:
                        for rt in range(4):
                            emit_v_group(C, xt, rt)

                for C in range(NCH):
                    xt_c = xt_first if C == 0 else emit_xt_dma(C)

                    # ---- attention for sq chunk C, both head pairs ----
                    # pair-1's projection is emitted after pair-0's attention
                    # so ACT exps overlap the remaining PE projection work
                    for p in range(2):
                        proj_pair(C, xt_c, p)
                        hA, hB = 2 * p, 2 * p + 1
                        pvA = ps_pv.tile([128, CHW], f32, tag="pv")
                        pvB = ps_pv.tile([128, CHW], f32, tag="pv")
                        nblk = 4 * C + 4

                        def emit_qk(i):
                            m = i - 4 * C
                            off = 0 if m < 0 else 128 * m
                            w = CHW - off
                            sqs = C * CHW + off
                            psM = ps_sc.tile([128, 2, CHW], f32, tag="sc")
                            nc.tensor.matmul(
                                psM[:, 0, 0:w],
                                kT2[0:64, p, i * 128:(i + 1) * 128],
                                qT2[0:64, p, sqs:sqs + w],
                                start=True, stop=True, tile_position=(0, 0))
                            nc.tensor.matmul(
                                psM[:, 1, 0:w],
                                kT2[64:128, p, i * 128:(i + 1) * 128],
                                qT2[64:128, p, sqs:sqs + w],
                                start=True, stop=True, tile_position=(64, 0))
                            return psM, m, off, w

                        def emit_tail(i, psM, m, off, w):
                            expM = exp_pool.tile([128, 2, CHW], bf16,
                                                 tag="exp")
                            if m >= 0:
                                # one strided ACT over both heads' valid cols
                                nc.scalar.activation(expM[:, :, 0:w],
                                                     psM[:, :, 0:w],
                                                     AF.Exp, scale=SCALE)
                                # causal mask: zero the upper triangle of the
                                # leading 128-col diagonal window (both heads)
                                nc.vector.tensor_mul(expM[:, :, 0:128],
                                                     expM[:, :, 0:128],
                                                     tri_sb[:])
                            else:
                                nc.scalar.activation(expM[:], psM[:],
                                                     AF.Exp, scale=SCALE)
                            nc.tensor.matmul(
                                pvA[0:65, off:CHW], v_sb[:, i, hA, :],
                                expM[:, 0, 0:w],
                                start=(i == 0), stop=(i == nblk - 1))
                            nc.tensor.matmul(
                                pvB[0:65, off:CHW], v_sb[:, i, hB, :],
                                expM[:, 1, 0:w],
                                start=(i == 0), stop=(i == nblk - 1))

                        pending = None
                        for i in range(nblk):
                            cur = emit_qk(i)
                            if pending is not None:
                                emit_tail(i - 1, *pending)
                            pending = cur
                        emit_tail(nblk - 1, *pending)
                        oA = out_pool.tile([128, CHW], f32, tag="o")
                        oB = out_pool.tile([128, CHW], f32, tag="o")
                        nc.vector.tensor_copy(oA[0:65, :], pvA[0:65, :])
                        nc.vector.tensor_copy(oB[0:65, :], pvB[0:65, :])
                        nc.sync.dma_start(OUT[hA, :, C * CHW:(C + 1) * CHW],
                                          oA[0:65, :])
                        nc.sync.dma_start(OUT[hB, :, C * CHW:(C + 1) * CHW],
                                          oB[0:65, :])

    nc.compile()
    return nc


def _get_nc(repeat=1):
    key = ("nc", repeat)
    if key not in _CACHE:
        _CACHE[key] = _build(repeat)
    return _CACHE[key]


def _tile_km(a):
    # [1024, N] -> [128, 8, N] partition-major over the 8 k-blocks
    return np.ascontiguousarray(
        a.reshape(KB, 128, a.shape[1]).transpose(1, 0, 2))


def _prep_inputs(x, W, b):
    import ml_dtypes
    bf16 = ml_dtypes.bfloat16

    x = np.asarray(x, dtype=np.float32)
    W = np.asarray(W, dtype=np.float32)
    b = np.asarray(b, dtype=np.float32)

    W4 = W.reshape(HIDDEN, 3, NUM_HEAD, HEAD_DIM)
    b4 = b.reshape(3, NUM_HEAD, HEAD_DIM)

    xT = [_tile_km(np.ascontiguousarray(x[bi].T)).astype(bf16)
          for bi in range(B)]

    tri = (np.arange(128)[None, :] >= np.arange(128)[:, None]).astype(
        np.float32)
    tri2 = np.concatenate([tri, tri], axis=1).astype(bf16)

    in_maps = []
    for c in range(N_CORES):
        bi, g = divmod(c, HPC)
        heads = [4 * g + j for j in range(HPC)]
        wqk = np.concatenate(
            [W4[:, 0, h, :] for h in heads] + [W4[:, 1, h, :] for h in heads],
            axis=1)  # [1024, 512]
        wv = np.concatenate([W4[:, 2, h, :] for h in heads], axis=1)  # [1024,256]
        bqkt = np.stack(
            [np.concatenate([b4[0, heads[0]], b4[0, heads[1]]]),
             np.concatenate([b4[0, heads[2]], b4[0, heads[3]]]),
             np.concatenate([b4[1, heads[0]], b4[1, heads[1]]]),
             np.concatenate([b4[1, heads[2]], b4[1, heads[3]]])],
            axis=1)  # [128, 4]
        in_maps.append({
            "XT": xT[bi],
            "WQK": _tile_km(np.ascontiguousarray(wqk)).astype(bf16),
            "WV": _tile_km(np.ascontiguousarray(wv)).astype(bf16),
            "BQKT": np.ascontiguousarray(bqkt),
            "TRI2": tri2,
        })
    return in_maps, b4


def kernel(x, W, b):
    from concourse.bass_utils import run_bass_kernel_spmd

    in_maps, b4 = _prep_inputs(x, W, b)
    nc = _get_nc()
    res = run_bass_kernel_spmd(nc, in_maps, core_ids=list(range(N_CORES)))

    out = np.empty((B, S, NUM_HEAD, HEAD_DIM), dtype=np.float32)
    for c in range(N_CORES):
        bi, g = divmod(c, HPC)
        u = res.results[c]["OUT"]               # [4, 65, 2048]
        o = u[:, :64, :] / u[:, 64:65, :]        # [4, 64, 2048]
        out[bi, :, 4 * g:4 * g + 4, :] = o.transpose(2, 0, 1)
    out += b4[2].reshape(1, 1, NUM_HEAD, HEAD_DIM)
    return out


# revision 13
# speedup vs baseline: 1.2904x; 1.2904x over previous
"""Distributed causal multi-head attention (QKV projection + flash attention)
for Trainium2, sharded head-parallel across 8 NeuronCores.

Problem: x[2,2048,1024] @ W[1024,3072] + b -> qkv; causal softmax attention
(16 heads, head_dim 64); output [2,2048,16,64].

Sharding: core c handles batch c//4 and the 4 heads 4*(c%4)..4*(c%4)+3.
Each core's output slice is disjoint -> no collectives.

Device kernel (per core, bf16 matmuls with fp32 PSUM accumulation):
  - host passes x pre-transposed AND pre-tiled (XT [128, 8, 2048] bf16,
    partition-major over the 8 k-blocks) and W column-sliced/reordered the
    same way, so every tensor loads with ONE large dma_start (single
    completion semaphore -> back-to-back matmul dispatch, no per-piece waits)
  - projection: qT/kT produced transposed ([head-pair 128, S]) with W as the
    stationary operand; v produced natural ([S,64] tiles) with xT stationary
  - attention per head-pair: scoresT[sk,sq] = kT.T @ qT row-packed 2 heads per
    PE pass (K=64 each, tile_position rows 0-63 / 64-127) into one 2-bank PSUM
    tile; exp on ACT with scale=1/8 (ONE op per block: full blocks use the
    contiguous [128,1024] range, diagonal blocks a strided [128,2,w] AP);
    causal via partial-width blocks + a bf16 0/1 triangular mask MULTIPLY
    (DVE, post-exp) on the diagonal 128-col window only; PV accumulates
    outT[65, sq] += v'[sk,65].T @ expT[sk,sq] where v' has a ones column
    (DVE memset) -> row 64 = softmax denominator.
  - output: unnormalized [4, 65, 2048] f32; host divides by row 64, adds the
    v bias, transposes into the full output.
"""

import numpy as np

NUM_HEAD = 16
HEAD_DIM = 64
HIDDEN = 1024
B, S = 2, 2048
N_CORES = 8
HPC = 4          # heads per core
NCH = 4          # sq chunks of 512
CHW = 512        # chunk width
NT = 16          # sk tiles of 128
KB = 8           # k-dim blocks of 128
SCALE = HEAD_DIM ** -0.5

_CACHE = {}


def _build(repeat=1):
    import concourse.bacc as bacc
    import concourse.mybir as mybir
    import concourse.tile as tile

    f32 = mybir.dt.float32
    bf16 = mybir.dt.bfloat16
    AF = mybir.ActivationFunctionType

    nc = bacc.Bacc("TRN2", target_bir_lowering=False, debug=False)

    XT = nc.dram_tensor("XT", [NCH, 128, KB, CHW], bf16, kind="ExternalInput")
    WQK = nc.dram_tensor("WQK", [128, KB, 512], bf16, kind="ExternalInput")
    WV = nc.dram_tensor("WV", [128, KB, 256], bf16, kind="ExternalInput")
    BQKT = nc.dram_tensor("BQKT", [128, 4], f32, kind="ExternalInput")
    TRI2 = nc.dram_tensor("TRI2", [128, 256], bf16, kind="ExternalInput")
    OUT = nc.dram_tensor("OUT", [HPC, 65, S], f32, kind="ExternalOutput")

    with tile.TileContext(nc) as tc:
        with tc.tile_pool(name="const", bufs=1) as const_pool, \
             tc.tile_pool(name="qkv", bufs=1) as qkv_pool, \
             tc.tile_pool(name="xt", bufs=4) as xt_pool, \
             tc.tile_pool(name="exps", bufs=8) as exp_pool, \
             tc.tile_pool(name="outs", bufs=4) as out_pool, \
             tc.tile_pool(name="ps_sc", bufs=2, space="PSUM") as ps_sc, \
             tc.tile_pool(name="ps_pr", bufs=2, space="PSUM") as ps_pr, \
             tc.tile_pool(name="ps_pv", bufs=2, space="PSUM") as ps_pv:

            for _rep in range(repeat):
                # ALL input DMAs on the sync HWDGE ring (FIFO execution per
                # ring) in priority order, so the chunk-0 halves the first
                # matmuls need don't share SDMA bandwidth with prefetches.
                # Output DMAs go on the gpsimd (SWDGE) ring instead.
                def emit_xt_dma(C):
                    lo = xt_pool.tile([128, KB // 2, CHW], bf16, tag="xt")
                    hi = xt_pool.tile([128, KB // 2, CHW], bf16, tag="xt")
                    nc.sync.dma_start(lo[:], XT[C, :, 0:KB // 2, :])
                    nc.sync.dma_start(hi[:], XT[C, :, KB // 2:KB, :])
                    return lo, hi

                wqk_lo = const_pool.tile([128, KB // 2, 512], bf16, tag="wqkl")
                wqk_hi = const_pool.tile([128, KB // 2, 512], bf16, tag="wqkh")
                wv_sb = const_pool.tile([128, KB, 256], bf16, tag="wv")
                bqk_sb = const_pool.tile([128, 4], f32, tag="bqk")
                tri_sb = const_pool.tile([128, 2, 128], bf16, tag="tri")

                nc.sync.dma_start(wqk_lo[:], WQK[:, 0:KB // 2, :])
                xt_first = emit_xt_dma(0)
                nc.sync.dma_start(wqk_hi[:], WQK[:, KB // 2:KB, :])
                nc.sync.dma_start(wv_sb[:], WV[:])
                nc.sync.dma_start(bqk_sb[:], BQKT[:])
                nc.sync.dma_start(tri_sb[:, 0, :], TRI2[:, 0:128])
                nc.sync.dma_start(tri_sb[:, 1, :], TRI2[:, 128:256])

                # qT2/kT2: [pair, 128 (2 heads x 64 d), S]; v: [sk-tile, head, 65]
                qT2 = qkv_pool.tile([128, 2, S], bf16, tag="qT2")
                kT2 = qkv_pool.tile([128, 2, S], bf16, tag="kT2")
                v_sb = qkv_pool.tile([128, NT, HPC, 65], bf16, tag="v")
                nc.vector.memset(v_sb[:, :, :, 64], 1.0)

                H = KB // 2

                def emit_qkT_group(C, xt, blk):
                    # col-blocks: 0,1 = q pair0/pair1; 2,3 = k pair0/pair1
                    ps = ps_pr.tile([128, CHW], f32, tag="pr")
                    for kb in range(KB):
                        w_sb = wqk_lo if kb < H else wqk_hi
                        nc.tensor.matmul(
                            ps[:],
                            w_sb[:, kb % H, blk * 128:(blk + 1) * 128],
                            xt[kb // H][:, kb % H, :],
                            start=(kb == 0), stop=(kb == KB - 1))
                    dest = (qT2 if blk < 2 else kT2)[:, blk % 2,
                                                     C * CHW:(C + 1) * CHW]
                    nc.vector.tensor_scalar_add(dest, ps[:],
                                                bqk_sb[:, blk:blk + 1])

                def emit_v_group(C, xt, rt):
                    t = C * 4 + rt
                    psv = ps_pr.tile([128, 256], f32, tag="pr")
                    for kb in range(KB):
                        nc.tensor.matmul(
                            psv[:],
                            xt[kb // H][:, kb % H, rt * 128:(rt + 1) * 128],
                            wv_sb[:, kb, :],
                            start=(kb == 0), stop=(kb == KB - 1))
                    nc.vector.tensor_copy(v_sb[:, t, :, 0:64], psv[:])

                def proj_pair(C, xt, p):
                    # groups needed by pair p's attention: q blk p, k blk 2+p,
                    # plus (for p==0) all v tiles of this chunk
                    emit_qkT_group(C, xt, p)
                    emit_qkT_group(C, xt, 2 + p)
                    if p == 0:
                        for rt in range(4):
                            emit_v_group(C, xt, rt)

                xt_tiles = [xt_first]
                for C in range(NCH):
                    if C + 1 < NCH:
                        xt_tiles.append(emit_xt_dma(C + 1))
                    xt_c = xt_tiles[C]

                    # pair-1's projection is emitted after pair-0's attention
                    # so ACT exps overlap the remaining PE projection work
                    for p in range(2):
                        proj_pair(C, xt_c, p)
                        hA, hB = 2 * p, 2 * p + 1
                        pvA = ps_pv.tile([128, CHW], f32, tag="pv")
                        pvB = ps_pv.tile([128, CHW], f32, tag="pv")
                        nblk = 4 * C + 4

                        def emit_qk(i):
                            m = i - 4 * C
                            off = 0 if m < 0 else 128 * m
                            w = CHW - off
                            sqs = C * CHW + off
                            psM = ps_sc.tile([128, 2, CHW], f32, tag="sc")
                            nc.tensor.matmul(
                                psM[:, 0, 0:w],
                                kT2[0:64, p, i * 128:(i + 1) * 128],
                                qT2[0:64, p, sqs:sqs + w],
                                start=True, stop=True, tile_position=(0, 0))
                            nc.tensor.matmul(
                                psM[:, 1, 0:w],
                                kT2[64:128, p, i * 128:(i + 1) * 128],
                                qT2[64:128, p, sqs:sqs + w],
                                start=True, stop=True, tile_position=(64, 0))
                            return psM, m, off, w

                        def emit_tail(i, psM, m, off, w):
                            expM = exp_pool.tile([128, 2, CHW], bf16,
                                                 tag="exp")
                            if m >= 0:
                                # one strided ACT over both heads' valid cols
                                nc.scalar.activation(expM[:, :, 0:w],
                                                     psM[:, :, 0:w],
                                                     AF.Exp, scale=SCALE)
                                # causal mask: zero the upper triangle of the
                                # leading 128-col diagonal window (both heads)
                                nc.vector.tensor_mul(expM[:, :, 0:128],
                                                     expM[:, :, 0:128],
                                                     tri_sb[:])
                            else:
                                nc.scalar.activation(expM[:], psM[:],
                                                     AF.Exp, scale=SCALE)
                            nc.tensor.matmul(
                                pvA[0:65, off:CHW], v_sb[:, i, hA, :],
                                expM[:, 0, 0:w],
                                start=(i == 0), stop=(i == nblk - 1))
                            nc.tensor.matmul(
                                pvB[0:65, off:CHW], v_sb[:, i, hB, :],
                                expM[:, 1, 0:w],
                                start=(i == 0), stop=(i == nblk - 1))

                        pending = None
                        for i in range(nblk):
                            cur = emit_qk(i)
                            if pending is not None:
                                emit_tail(i - 1, *pending)
                            pending = cur
                        emit_tail(nblk - 1, *pending)
                        oA = out_pool.tile([128, CHW], f32, tag="o")
                        oB = out_pool.tile([128, CHW], f32, tag="o")
                        nc.vector.tensor_copy(oA[0:65, :], pvA[0:65, :])
                        nc.vector.tensor_copy(oB[0:65, :], pvB[0:65, :])
                        nc.gpsimd.dma_start(OUT[hA, :, C * CHW:(C + 1) * CHW],
                                            oA[0:65, :])
                        nc.gpsimd.dma_start(OUT[hB, :, C * CHW:(C + 1) * CHW],
                                            oB[0:65, :])

    nc.compile()
    return nc


def _get_nc(repeat=1):
    key = ("nc", repeat)
    if key not in _CACHE:
        _CACHE[key] = _build(repeat)
    return _CACHE[key]


def _tile_km(a):
    # [1024, N] -> [128, 8, N] partition-major over the 8 k-blocks
    return np.ascontiguousarray(
        a.reshape(KB, 128, a.shape[1]).transpose(1, 0, 2))


def _prep_inputs(x, W, b):
    import ml_dtypes
    bf16 = ml_dtypes.bfloat16

    x = np.asarray(x, dtype=np.float32)
    W = np.asarray(W, dtype=np.float32)
    b = np.asarray(b, dtype=np.float32)

    W4 = W.reshape(HIDDEN, 3, NUM_HEAD, HEAD_DIM)
    b4 = b.reshape(3, NUM_HEAD, HEAD_DIM)

    # [128, KB, S] -> per-chunk contiguous [NCH, 128, KB, CHW]
    xT = [np.ascontiguousarray(
        _tile_km(np.ascontiguousarray(x[bi].T))
        .reshape(128, KB, NCH, CHW).transpose(2, 0, 1, 3)).astype(bf16)
        for bi in range(B)]

    tri = (np.arange(128)[None, :] >= np.arange(128)[:, None]).astype(
        np.float32)
    tri2 = np.concatenate([tri, tri], axis=1).astype(bf16)

    in_maps = []
    for c in range(N_CORES):
        bi, g = divmod(c, HPC)
        heads = [4 * g + j for j in range(HPC)]
        wqk = np.concatenate(
            [W4[:, 0, h, :] for h in heads] + [W4[:, 1, h, :] for h in heads],
            axis=1)  # [1024, 512]
        wv = np.concatenate([W4[:, 2, h, :] for h in heads], axis=1)  # [1024,256]
        bqkt = np.stack(
            [np.concatenate([b4[0, heads[0]], b4[0, heads[1]]]),
             np.concatenate([b4[0, heads[2]], b4[0, heads[3]]]),
             np.concatenate([b4[1, heads[0]], b4[1, heads[1]]]),
             np.concatenate([b4[1, heads[2]], b4[1, heads[3]]])],
            axis=1)  # [128, 4]
        in_maps.append({
            "XT": xT[bi],
            "WQK": _tile_km(np.ascontiguousarray(wqk)).astype(bf16),
            "WV": _tile_km(np.ascontiguousarray(wv)).astype(bf16),
            "BQKT": np.ascontiguousarray(bqkt),
            "TRI2": tri2,
        })
    return in_maps, b4


def kernel(x, W, b):
    from concourse.bass_utils import run_bass_kernel_spmd

    in_maps, b4 = _prep_inputs(x, W, b)
    nc = _get_nc()
    res = run_bass_kernel_spmd(nc, in_maps, core_ids=list(range(N_CORES)))

    out = np.empty((B, S, NUM_HEAD, HEAD_DIM), dtype=np.float32)
    for c in range(N_CORES):
        bi, g = divmod(c, HPC)
        u = res.results[c]["OUT"]               # [4, 65, 2048]
        o = u[:, :64, :] / u[:, 64:65, :]        # [4, 64, 2048]
        out[bi, :, 4 * g:4 * g + 4, :] = o.transpose(2, 0, 1)
    out += b4[2].reshape(1, 1, NUM_HEAD, HEAD_DIM)
    return out
